# revision 4
# baseline (speedup 1.0000x reference)
"""Trainium2 Bass kernel for nn_CVEncoder (histogram_binning).

Pipeline (reference semantics):
  1. Per curve (M = BS*K = 512): np.interp of velocity picks at H=256 time
     samples -> vq, vIdx = clip(round(vq), 0, 255).
  2. soft[m] = 0.01 + 0.9 * one_hot(vIdx[m])        (256 x 256 image)
  3. out[m] = bilinear-resize soft along H: 256 -> 512 (W unchanged:
     half-pixel centers make the W-resize an exact identity).

The W-identity + 2x H upsample collapse to: every output row is a fixed
linear combination (weights {0.25, 0.75, 1.0}) of at most two adjacent
soft rows.  So per curve:

    OUT (512x256) = A (512x256, banded) @ onehot(vIdx) (256x256)
    out           = 0.9 * OUT + 0.01

which the device computes as: DVE builds one-hot rows via is_equal
against an iota row (bf16, all values exact), TensorE applies the banded
upsample matrix (bf16 weights 0.25/0.75/1.0, all exact -> fp32 PSUM is
exact), ScalarE fuses the 0.9 scale + 0.01 background bias on the
PSUM->SBUF copy, and DMA streams the 32 MB/core result to HBM.  The
kernel is memory-bound on the output write (~90 us/core roofline).

vIdx itself is computed on host in numpy: it needs three IEEE f32
divisions reproduced bit-exactly (the fixed dataset contains a vq that
lands *exactly* on a .5 rounding boundary, so any 1-ulp deviation flips
a histogram bin).  The TRN2 vector engines have no float-divide ALU op
(tensor_tensor/tensor_scalar `divide` fail the ISA check), so the exact
computation cannot be expressed on device; it is 131K elements of prep
vs 67M elements of output.

Sharding: embarrassingly data-parallel over BS — batches 2i, 2i+1
(64 curves) per core i, no cross-core communication.
"""

import numpy as np
import ml_dtypes

import concourse.bacc as bacc
import concourse.mybir as mybir
from concourse import tile
from concourse.bass_utils import run_bass_kernel_spmd

# problem constants (hardcoded per contract)
T0, T1 = 0.0, 7000.0
H, W = 256, 256
RH, RW = 512, 256
BS, K, N = 16, 32, 12
M = BS * K
N_CORES = 8
CURVES_PER_CORE = M // N_CORES  # 64
# soft-row window start per output row-block tau (out rows 128*tau..+127
# need soft rows [64*tau - 1, 64*tau + 64], all inside [s, s+128))
S_TAU = (0, 63, 127, 128)

BF16 = ml_dtypes.bfloat16


def _compute_vidx(VelPoints, VMM):
    """Bit-exact numpy replication of the reference interp -> vIdx (int32 [M, H])."""
    t = np.ascontiguousarray(VelPoints[..., 0], dtype=np.float32)
    v = np.ascontiguousarray(VelPoints[..., 1], dtype=np.float32)
    VMM = np.asarray(VMM, dtype=np.float32)
    dt = np.float32((T1 - T0) / (H - 1))
    tn = (t - np.float32(T0)) / dt
    dv = (VMM[:, 1] - VMM[:, 0]) / np.float32(W - 1)
    vn = (v - VMM[:, 0][:, None, None]) / dv[:, None, None]
    mask = tn > 0
    tn = tn.reshape(M, N)
    vn = vn.astype(np.float32).reshape(M, N)
    mask = mask.reshape(M, N)

    xp = np.where(mask, tn, np.float32(np.inf))
    order = np.argsort(xp, axis=1, kind="stable")
    xp = np.take_along_axis(xp, order, 1)
    fp = np.take_along_axis(vn, order, 1)
    nvalid = mask.sum(axis=1)

    q = np.arange(H, dtype=np.float32)
    ss = np.empty((M, H), dtype=np.int64)
    for m in range(M):
        ss[m] = np.searchsorted(xp[m], q, side="right")
    hi = np.clip(ss, 1, np.maximum(nvalid - 1, 1)[:, None])
    lo = hi - 1
    x0 = np.take_along_axis(xp, lo, 1)
    x1 = np.take_along_axis(xp, hi, 1)
    y0 = np.take_along_axis(fp, lo, 1)
    y1 = np.take_along_axis(fp, hi, 1)
    denom = x1 - x0
    safe = np.where(denom > 0, denom, np.float32(1.0)).astype(np.float32)
    val = (y0 + (q[None, :] - x0) / safe * (y1 - y0)).astype(np.float32)
    last = np.maximum(nvalid - 1, 0)[:, None]
    xlast = np.take_along_axis(xp, last, 1)
    ylast = np.take_along_axis(fp, last, 1)
    val = np.where(q[None, :] <= xp[:, :1], fp[:, :1], val)
    val = np.where(q[None, :] >= xlast, ylast, val).astype(np.float32)
    return np.clip(np.round(val), 0, W - 1).astype(np.int32)


def _build_upsample_weights():
    """lhsT weight mats [4][128k, 128p]: out row 128*tau+p = sum_k W[tau,k,p] * soft[S_TAU[tau]+k]."""
    wts = np.zeros((4, 128, 128), dtype=np.float32)
    for tau in range(4):
        s = S_TAU[tau]
        for p in range(128):
            r = 128 * tau + p
            j = r >> 1
            if r % 2 == 0:
                pairs = ((max(j - 1, 0), 0.25), (j, 0.75))
            else:
                pairs = ((j, 0.75), (min(j + 1, H - 1), 0.25))
            for m, w in pairs:
                k = m - s
                assert 0 <= k < 128
                wts[tau, k, p] += w
    return wts


_COMPILED = None


def _get_module():
    """Build (once) the SPMD Bass module for one core's 64 curves."""
    global _COMPILED
    if _COMPILED is not None:
        return _COMPILED

    nc = bacc.Bacc(None, target_bir_lowering=False)
    bf = mybir.dt.bfloat16
    f32 = mybir.dt.float32

    vt_d = nc.dram_tensor("vt", (128, 4, CURVES_PER_CORE), f32, kind="ExternalInput")
    iota_d = nc.dram_tensor("iota", (128, W), bf, kind="ExternalInput")
    wts_d = nc.dram_tensor("wts", (128, 4, 128), bf, kind="ExternalInput")
    out_d = nc.dram_tensor("out", (CURVES_PER_CORE, RH, RW), f32, kind="ExternalOutput")

    with tile.TileContext(nc) as tc:
        with (
            tc.tile_pool(name="const", bufs=1) as cpool,
            tc.tile_pool(name="work", bufs=8) as wpool,
            tc.tile_pool(name="psum", bufs=6, space="PSUM") as ppool,
            tc.tile_pool(name="outp", bufs=12) as opool,
        ):
            vt = cpool.tile([128, 4, CURVES_PER_CORE], f32)
            nc.sync.dma_start(vt[:], vt_d[:])
            bias = cpool.tile([128, 1], f32)
            nc.vector.memset(bias[:], 0.01)
            iota = cpool.tile([128, W], bf)
            nc.sync.dma_start(iota[:], iota_d[:])
            wts = cpool.tile([128, 4, 128], bf)
            nc.sync.dma_start(wts[:], wts_d[:])

            for tau in range(4):
                for p2 in range(CURVES_PER_CORE // 2):
                    c0, c1 = 2 * p2, 2 * p2 + 1
                    e = wpool.tile([128, 2 * W], bf, name="e")
                    nc.vector.tensor_scalar(
                        e[:, 0:W], iota[:], vt[:, tau, c0 : c0 + 1], None,
                        mybir.AluOpType.is_equal,
                    )
                    nc.vector.tensor_scalar(
                        e[:, W : 2 * W], iota[:], vt[:, tau, c1 : c1 + 1], None,
                        mybir.AluOpType.is_equal,
                    )
                    ps = ppool.tile([128, 2 * W], f32, name="ps")
                    nc.tensor.matmul(ps[:], wts[:, tau, :], e[:])
                    ob = opool.tile([128, 2 * W], f32, name="ob")
                    nc.scalar.activation(
                        ob[:], ps[:], mybir.ActivationFunctionType.Identity,
                        bias=bias[:], scale=0.9,
                    )
                    rows = slice(128 * tau, 128 * (tau + 1))
                    nc.sync.dma_start(out_d[c0, rows, :], ob[:, 0:W])
                    nc.sync.dma_start(out_d[c1, rows, :], ob[:, W : 2 * W])

    nc.compile()

    iota_np = np.broadcast_to(np.arange(W, dtype=np.float32), (128, W)).astype(BF16)
    wts_np = _build_upsample_weights().transpose(1, 0, 2).astype(BF16)  # [128,4,128]
    wts_np = np.ascontiguousarray(wts_np)
    _COMPILED = (nc, iota_np, wts_np)
    return _COMPILED


def kernel(VelPoints, VMM):
    vidx = _compute_vidx(VelPoints, VMM)  # [M, H] int32

    nc, iota_np, wts_np = _get_module()

    # per-core vt[p, tau, c] = vIdx[core*64 + c, S_TAU[tau] + p], bf16 (exact ints)
    in_maps = []
    for core in range(N_CORES):
        vloc = vidx[core * CURVES_PER_CORE : (core + 1) * CURVES_PER_CORE]  # [64, 256]
        vt = np.empty((128, 4, CURVES_PER_CORE), dtype=np.float32)
        for tau in range(4):
            s = S_TAU[tau]
            vt[:, tau, :] = vloc[:, s : s + 128].T
        in_maps.append({"vt": vt, "iota": iota_np, "wts": wts_np})

    res = run_bass_kernel_spmd(nc, in_maps, core_ids=list(range(N_CORES)))
    out = np.concatenate(
        [r["out"].reshape(2, K, RH, RW) for r in res.results], axis=0
    )
    return out


# revision 7
# speedup vs baseline: 1.5858x; 1.5858x over previous
"""Trainium2 Bass kernel for nn_CVEncoder (histogram_binning).

Pipeline (reference semantics):
  1. Per curve (M = BS*K = 512): np.interp of velocity picks at H=256 time
     samples -> vq, vIdx = clip(round(vq), 0, 255).
  2. soft[m] = 0.01 + 0.9 * one_hot(vIdx[m])        (256 x 256 image)
  3. out[m] = bilinear-resize soft along H: 256 -> 512 (W unchanged:
     half-pixel centers make the W-resize an exact identity).

The W-identity + 2x H upsample collapse to: every output row is a fixed
linear combination (weights {0.25, 0.75, 1.0}) of at most two adjacent
soft rows.  So per curve:

    OUT (512x256) = A (512x256, banded) @ onehot(vIdx) (256x256)
    out           = 0.9 * OUT + 0.01

which the device computes as: DVE builds one-hot rows via is_equal
against an iota row (bf16, all values exact), TensorE applies the banded
upsample matrix (bf16 weights 0.25/0.75/1.0, all exact -> fp32 PSUM is
exact), ScalarE fuses the 0.9 scale + 0.01 background bias on the
PSUM->SBUF copy, and DMA streams the 32 MB/core result to HBM.  The
kernel is memory-bound on the output write (~90 us/core roofline).

vIdx itself is computed on host in numpy: it needs three IEEE f32
divisions reproduced bit-exactly (the fixed dataset contains a vq that
lands *exactly* on a .5 rounding boundary, so any 1-ulp deviation flips
a histogram bin).  The TRN2 vector engines have no float-divide ALU op
(tensor_tensor/tensor_scalar `divide` fail the ISA check), so the exact
computation cannot be expressed on device; it is 131K elements of prep
vs 67M elements of output.

Sharding: embarrassingly data-parallel over BS — batches 2i, 2i+1
(64 curves) per core i, no cross-core communication.
"""

import numpy as np
import ml_dtypes

import concourse.bacc as bacc
import concourse.mybir as mybir
from concourse import tile
from concourse.bass_utils import run_bass_kernel_spmd

# problem constants (hardcoded per contract)
T0, T1 = 0.0, 7000.0
H, W = 256, 256
RH, RW = 512, 256
BS, K, N = 16, 32, 12
M = BS * K
N_CORES = 8
CURVES_PER_CORE = M // N_CORES  # 64
# soft-row window start per output row-block tau (out rows 128*tau..+127
# need soft rows [64*tau - 1, 64*tau + 64], all inside [s, s+128))
S_TAU = (0, 63, 127, 128)

BF16 = ml_dtypes.bfloat16


def _compute_vidx(VelPoints, VMM):
    """Bit-exact numpy replication of the reference interp -> vIdx (int32 [M, H])."""
    t = np.ascontiguousarray(VelPoints[..., 0], dtype=np.float32)
    v = np.ascontiguousarray(VelPoints[..., 1], dtype=np.float32)
    VMM = np.asarray(VMM, dtype=np.float32)
    dt = np.float32((T1 - T0) / (H - 1))
    tn = (t - np.float32(T0)) / dt
    dv = (VMM[:, 1] - VMM[:, 0]) / np.float32(W - 1)
    vn = (v - VMM[:, 0][:, None, None]) / dv[:, None, None]
    mask = tn > 0
    tn = tn.reshape(M, N)
    vn = vn.astype(np.float32).reshape(M, N)
    mask = mask.reshape(M, N)

    xp = np.where(mask, tn, np.float32(np.inf))
    order = np.argsort(xp, axis=1, kind="stable")
    xp = np.take_along_axis(xp, order, 1)
    fp = np.take_along_axis(vn, order, 1)
    nvalid = mask.sum(axis=1)

    q = np.arange(H, dtype=np.float32)
    ss = np.empty((M, H), dtype=np.int64)
    for m in range(M):
        ss[m] = np.searchsorted(xp[m], q, side="right")
    hi = np.clip(ss, 1, np.maximum(nvalid - 1, 1)[:, None])
    lo = hi - 1
    x0 = np.take_along_axis(xp, lo, 1)
    x1 = np.take_along_axis(xp, hi, 1)
    y0 = np.take_along_axis(fp, lo, 1)
    y1 = np.take_along_axis(fp, hi, 1)
    denom = x1 - x0
    safe = np.where(denom > 0, denom, np.float32(1.0)).astype(np.float32)
    val = (y0 + (q[None, :] - x0) / safe * (y1 - y0)).astype(np.float32)
    last = np.maximum(nvalid - 1, 0)[:, None]
    xlast = np.take_along_axis(xp, last, 1)
    ylast = np.take_along_axis(fp, last, 1)
    val = np.where(q[None, :] <= xp[:, :1], fp[:, :1], val)
    val = np.where(q[None, :] >= xlast, ylast, val).astype(np.float32)
    return np.clip(np.round(val), 0, W - 1).astype(np.int32)


def _build_upsample_weights():
    """lhsT weight mats [4][128k, 128p]: out row 128*tau+p = sum_k W[tau,k,p] * soft[S_TAU[tau]+k]."""
    wts = np.zeros((4, 128, 128), dtype=np.float32)
    for tau in range(4):
        s = S_TAU[tau]
        for p in range(128):
            r = 128 * tau + p
            j = r >> 1
            if r % 2 == 0:
                pairs = ((max(j - 1, 0), 0.25), (j, 0.75))
            else:
                pairs = ((j, 0.75), (min(j + 1, H - 1), 0.25))
            for m, w in pairs:
                k = m - s
                assert 0 <= k < 128
                wts[tau, k, p] += w
    return wts


_COMPILED = None


def _get_module():
    """Build (once) the SPMD Bass module for one core's 64 curves."""
    global _COMPILED
    if _COMPILED is not None:
        return _COMPILED

    nc = bacc.Bacc(None, target_bir_lowering=False)
    bf = mybir.dt.bfloat16
    f32 = mybir.dt.float32

    vt_d = nc.dram_tensor("vt", (128, 4, CURVES_PER_CORE), f32, kind="ExternalInput")
    iota_d = nc.dram_tensor("iota", (128, W), bf, kind="ExternalInput")
    wts_d = nc.dram_tensor("wts", (128, 4, 128), bf, kind="ExternalInput")
    out_d = nc.dram_tensor("out", (CURVES_PER_CORE, RH, RW), f32, kind="ExternalOutput")

    with tile.TileContext(nc) as tc:
        with (
            tc.tile_pool(name="const", bufs=1) as cpool,
            tc.tile_pool(name="work", bufs=8) as wpool,
            tc.tile_pool(name="psum", bufs=4, space="PSUM") as ppool,
            tc.tile_pool(name="outp", bufs=3) as opool,
        ):
            vt = cpool.tile([128, 4, CURVES_PER_CORE], f32)
            nc.sync.dma_start(vt[:], vt_d[:])
            bias = cpool.tile([128, 1], f32)
            nc.vector.memset(bias[:], 0.01)
            iota = cpool.tile([128, W], bf)
            nc.sync.dma_start(iota[:], iota_d[:])
            wts = cpool.tile([128, 4, 128], bf)
            nc.sync.dma_start(wts[:], wts_d[:])

            # per pair of curves (c0, c1): build the full [2 x 512 x 256] output
            # block in SBUF (ob free layout = [tau(4), c(2), w(256)]), then one
            # 1 MB DMA to the contiguous DRAM span of the two curves.
            # DMAs alternate between the SP HWDGE ring (nc.sync) and the SWDGE
            # path (nc.gpsimd) so neither descriptor path serializes the kernel.
            for p2 in range(CURVES_PER_CORE // 2):
                c0, c1 = 2 * p2, 2 * p2 + 1
                ob = opool.tile([128, 4, 2, W], f32, name="ob")
                for th in range(2):  # tau pair {2*th, 2*th+1} -> one 2-bank PSUM
                    ps = ppool.tile([128, 2, 2, W], f32, name="ps")
                    for i in range(2):
                        tau = 2 * th + i
                        e = wpool.tile([128, 2 * W], bf, name="e")
                        nc.vector.tensor_scalar(
                            e[:, 0:W], iota[:], vt[:, tau, c0 : c0 + 1], None,
                            mybir.AluOpType.is_equal,
                        )
                        nc.vector.tensor_scalar(
                            e[:, W : 2 * W], iota[:], vt[:, tau, c1 : c1 + 1], None,
                            mybir.AluOpType.is_equal,
                        )
                        nc.tensor.matmul(ps[:, i, :, :], wts[:, tau, :], e[:])
                    nc.scalar.activation(
                        ob[:, 2 * th : 2 * th + 2, :, :], ps[:],
                        mybir.ActivationFunctionType.Identity,
                        bias=bias[:], scale=0.9,
                    )
                # one 512 KB DMA per curve on the SP HWDGE ring (SWDGE/gpsimd
                # DMAs here crashed the device - likely the DVE 2-port perf
                # mode vs SWDGE descriptor-ring SBUF contention)
                for ci, c in ((0, c0), (1, c1)):
                    dst = out_d[c].rearrange("(t p) w -> p t w", t=4)
                    nc.sync.dma_start(dst, ob[:, :, ci, :])

    nc.compile()

    iota_np = np.broadcast_to(np.arange(W, dtype=np.float32), (128, W)).astype(BF16)
    wts_np = _build_upsample_weights().transpose(1, 0, 2).astype(BF16)  # [128,4,128]
    wts_np = np.ascontiguousarray(wts_np)
    _COMPILED = (nc, iota_np, wts_np)
    return _COMPILED


def kernel(VelPoints, VMM):
    vidx = _compute_vidx(VelPoints, VMM)  # [M, H] int32

    nc, iota_np, wts_np = _get_module()

    # per-core vt[p, tau, c] = vIdx[core*64 + c, S_TAU[tau] + p], bf16 (exact ints)
    in_maps = []
    for core in range(N_CORES):
        vloc = vidx[core * CURVES_PER_CORE : (core + 1) * CURVES_PER_CORE]  # [64, 256]
        vt = np.empty((128, 4, CURVES_PER_CORE), dtype=np.float32)
        for tau in range(4):
            s = S_TAU[tau]
            vt[:, tau, :] = vloc[:, s : s + 128].T
        in_maps.append({"vt": vt, "iota": iota_np, "wts": wts_np})

    res = run_bass_kernel_spmd(nc, in_maps, core_ids=list(range(N_CORES)))
    out = np.concatenate(
        [r["out"].reshape(2, K, RH, RW) for r in res.results], axis=0
    )
    return out
